# revision 39
# baseline (speedup 1.0000x reference)
"""Locally-connected conv (unshared weights) on 8 TRN2 NeuronCores.

Problem: inputs [64,32,32,64] f32, kernel [32,32,576,64] f32 (per-location
weights, KFEAT=3*3*64), bias [32,32,64] f32 -> out [64,32,32,64] f32
(SAME padding, stride 1).

Strategy (X-stationary, fp8e3 weight-streaming):
  - Spatial shard: core c computes output rows 4c..4c+3 (host slices the
    zero-padded input with halo; no device collectives needed).
  - Weights are unshared -> each weight element is used exactly once, so
    they are the *moving* matmul operand, streamed from HBM as fp8e3
    (e3m4) scaled by 2^5; X is bf16 scaled by 2^-5 so the product is at
    natural scale (measured end-to-end rel err ~1.35e-2 vs the 2e-2
    gate). This halves the dominant HBM traffic vs bf16 (9.2->4.6MB).
  - X patches are the *stationary* operand. K packs 2 input rows x 64
    channels = 128. xp holds only the 32 real columns (no pad cols).
  - Per output row pair (i0=2g, i1=2g+1) and input column c:
      M1: x pair g     (rows i0,i0+1), K=128 -> row i0 kh=(0,1)
      M2: x pair g+1   (rows i0+2,i0+3), K=128 -> row i1 kh=(1,2)
      MS: merged singles: block-diagonal stationary xs_g [128,128]
          (upper-left = row i0+2 for row i0's kh=2; lower-right =
          row i0+1 for row i1's kh=0; zeros elsewhere) -> ONE K=128
          matmul streams both rows' single-tap weights (two K=64
          matmuls would double-stream these columns through the PE).
    xs tiles are built on-device by the vector engine (memset + two
    strided copies from xp) while the first weight chunks stream in.
  - PSUM: bank tile [128,512] holds both rows of a group: even row on
    partitions 0:64, odd row on 64:128. All 4 output rows in 8 banks.
  - Bias is added with one K=2 bf16 matmul per PSUM bank (lhsT = parity
    indicator matrix) which also initializes the bank (start=True).
  - The whole fp8 weight stream (36KB/partition) is SBUF-resident:
    every chunk has its own buffer and all DMAs issue up front on the
    two HWDGE rings (sync/scalar), so the SDMA engines never starve;
    xp rides the gpsimd SWDGE ring in parallel.
"""

import numpy as np
import ml_dtypes

import concourse.bass as bass  # noqa: F401
import concourse.mybir as mybir
import concourse.tile as tile
from concourse import bacc
from concourse.bass_utils import run_bass_kernel_spmd

BF16 = ml_dtypes.bfloat16
E3M4 = ml_dtypes.float8_e3m4

B, H, W, CIN, COUT = 64, 32, 32, 64, 64
KH, KW = 3, 3
KFEAT = KH * KW * CIN
NCORES = 8
RPC = H // NCORES              # output rows per core = 4
HP, WP = H + 2, W + 2          # zero-padded input dims
NPAIRS = 3                     # input row pairs per core (6 padded rows)
PAIR_COLS = W * B              # 2048 free cols per pair tile (real cols only)
XP_COLS = NPAIRS * PAIR_COLS   # 6144
GROUPS = 2                     # output row pairs per core
BANKS = 4                      # psum banks per group
JPB = 8                        # output cols per bank (512 f32 / 64 co)
ROW_COLS = W * COUT            # 2048
XS_COLS = W * 128              # 4096: per-c [128,128] block-diag stationary
WSCALE = 32.0                  # weights *= 2^5 (fp8e3), x *= 2^-5
# bias stream [2, BS_COLS]: cols 0:128 = parity indicator matrix
# (row0 = 1 on m<64, row1 = 1 on m>=64), then per-(g,bank) [2,512] bias
# blocks (row0 = even-row bias, row1 = odd-row bias).
BS_COLS = 128 + GROUPS * BANKS * 512  # 4224


def stream_layout():
    """Weight stream block order. Returns (records, chunks, total_cols).

    record = (g, c, typ, jset, col_off); typ 0=M1, 1=M2, 2=MS(merged).
    chunks = list of (start_col, end_col), broken at (g,c) boundaries.
    """
    recs = []
    off = 0
    bounds = []
    # Order constraints: (a) consecutive matmuls must not accumulate
    # into overlapping psum elements (adjacent c's share output
    # columns; back-to-back overlapping accumulates stall the PE on
    # the drain) -- solved by the c zigzag 1,17,2,18,... within a
    # phase and by interleaving g0/g1 in the singles phase; (b) the
    # stream opens on M1-g0 alone so only x pair 0 must have arrived
    # for the first weight matmul (pair 1 gates A2/B1, pair 2 only
    # B2 at ~75% of the stream).
    zig = [c for cc in zip(range(1, W // 2 + 1),
                           range(W // 2 + 1, W + 1)) for c in cc]
    for g in range(GROUPS):
        for typ in (0, 1):
            for c in zig:
                jset = [j for j in (c - 2, c - 1, c) if 0 <= j < W]
                recs.append((g, c, typ, jset, off))
                off += 64 * len(jset)
                bounds.append(off)
    for c in range(1, WP - 1):
        jset = [j for j in (c - 2, c - 1, c) if 0 <= j < W]
        for g in range(GROUPS):
            recs.append((g, c, 2, jset, off))
            off += 64 * len(jset)
        bounds.append(off)
    chunks = []
    start, prev = 0, 0
    for b_ in bounds:
        cap = (1152, 2304)[len(chunks)] if len(chunks) < 2 else 4608
        if b_ - start > cap:
            chunks.append((start, prev))
            start = prev
        prev = b_
    chunks.append((start, prev))
    return recs, chunks, off


_RECS, _CHUNKS, TOTAL_COLS = stream_layout()


def mm_records():
    """Expand stream records into per-matmul records with psum targets."""
    chunk_of = {}
    for k, (a, b_) in enumerate(_CHUNKS):
        for g, c, typ, jset, off in _RECS:
            if a <= off < b_:
                chunk_of[off] = k
    mms = []
    for g, c, typ, jset, off in _RECS:
        # split jset (contiguous ascending) into per-bank pieces
        s = 0
        while s < len(jset):
            bk = jset[s] // JPB
            e = s
            while e < len(jset) and jset[e] // JPB == bk:
                e += 1
            c0 = off + s * 64
            c1 = off + e * 64
            o0 = (jset[s] % JPB) * 64
            o1 = o0 + (e - s) * 64
            if typ == 0:    # M1: row i0 (par 0), x pair g, K=128
                src, xoff, par = "xp", g * PAIR_COLS + (c - 1) * 64, 0
            elif typ == 1:  # M2: row i1 (par 1), x pair g+1, K=128
                src, xoff, par = "xp", (g + 1) * PAIR_COLS + (c - 1) * 64, 1
            else:           # MS: merged singles, block-diag xs_g, K=128
                src, xoff, par = "xs", (c - 1) * 128, 2
            mms.append(dict(g=g, bk=bk, src=src, xoff=xoff, par=par,
                            c0=c0, c1=c1, o0=o0, o1=o1,
                            chunk=chunk_of[off]))
            s = e
    return mms


_weight_template_cache = [None]


def weight_template():
    """int64 [128, TOTAL_COLS]: flat index into core-0 kernel array."""
    if _weight_template_cache[0] is not None:
        return _weight_template_cache[0]
    T = np.empty((128, TOTAL_COLS), np.int64)
    co = np.arange(COUT)
    p = np.arange(128)
    ci = p % 64
    for g, c, typ, jset, off in _RECS:
        for jj, j in enumerate(jset):
            kw = c - j
            if typ == 0:
                i = np.full(128, 2 * g)
                kh = np.where(p < 64, 0, 1)
            elif typ == 1:
                i = np.full(128, 2 * g + 1)
                kh = np.where(p < 64, 1, 2)
            else:
                i = np.where(p < 64, 2 * g, 2 * g + 1)
                kh = np.where(p < 64, 2, 0)
            # conv_general_dilated_local flattens KFEAT as (ci, kh, kw)
            kf = ci * (KH * KW) + kh * KW + kw
            base = ((i * W + j) * KFEAT + kf) * COUT
            T[:, off + jj * 64: off + (jj + 1) * 64] = base[:, None] + co[None, :]
    _weight_template_cache[0] = T
    return T


def prep_in_maps(inputs, kernel, bias):
    inputs = np.asarray(inputs, np.float32)
    kernel = np.asarray(kernel, np.float32)
    bias = np.asarray(bias, np.float32)
    T = weight_template()
    kflat = np.ascontiguousarray(kernel).reshape(-1)
    xpad = np.zeros((B, HP, W, CIN), np.float32)
    xpad[:, 1:H + 1, :, :] = inputs * (1.0 / WSCALE)
    xpad = xpad.astype(BF16)
    in_maps = []
    for core in range(NCORES):
        rows = xpad[:, RPC * core: RPC * core + 6]          # [B, 6, W, CIN]
        rt = rows.transpose(1, 3, 2, 0)                     # [r, ci, col, b]
        rt = rt.reshape(NPAIRS, 2, CIN, W, B).transpose(1, 2, 0, 3, 4)
        xp = np.ascontiguousarray(rt.reshape(128, XP_COLS))  # [rip*ci, rp,col,b]
        woff = (RPC * core) * W * KFEAT * COUT
        wt = np.clip(kflat[T + woff] * WSCALE, -15.5, 15.5).astype(E3M4)
        wt = np.concatenate([wt[:, a:b].reshape(-1) for a, b in _CHUNKS])
        bsh = bias[RPC * core: RPC * core + RPC].reshape(2 * GROUPS, ROW_COLS)
        bs = np.zeros((2, BS_COLS), np.float32)
        bs[0, 0:64] = 1.0
        bs[1, 64:128] = 1.0
        for g in range(GROUPS):
            for bk in range(BANKS):
                a = 128 + (g * BANKS + bk) * 512
                bs[0, a:a + 512] = bsh[2 * g, bk * 512:(bk + 1) * 512]
                bs[1, a:a + 512] = bsh[2 * g + 1, bk * 512:(bk + 1) * 512]
        in_maps.append({"xp": xp, "wt": wt, "bs": bs.astype(BF16)})
    return in_maps


def build_nc():
    dt = mybir.dt
    nc = bacc.Bacc(None, target_bir_lowering=False, debug=False)
    xp_d = nc.declare_dram_parameter("xp", [128, XP_COLS], dt.bfloat16,
                                     isOutput=False)
    wt_d = nc.declare_dram_parameter("wt", [128 * TOTAL_COLS], dt.float8e3,
                                     isOutput=False)
    bs_d = nc.declare_dram_parameter("bs", [2, BS_COLS], dt.bfloat16,
                                     isOutput=False)
    out_d = nc.declare_dram_parameter("out", [GROUPS, BANKS, 128, 512],
                                      dt.bfloat16, isOutput=True)

    mms = mm_records()
    for m in mms:
        m["stop"] = False
    last_bk = {}
    for idx, m in enumerate(mms):
        last_bk[(m["g"], m["bk"])] = idx
    for idx in last_bk.values():
        mms[idx]["stop"] = True
    evac_after = {idx: key for key, idx in last_bk.items()}

    with tile.TileContext(nc) as tc:
        with tc.tile_pool(name="const", bufs=1) as cpool, \
             tc.tile_pool(name="ps", bufs=1, space="PSUM") as pspool:
            bs_t = cpool.tile([2, BS_COLS], dt.bfloat16, name="bs_t",
                              tag="bs_t")
            nc.scalar.dma_start(out=bs_t[:], in_=bs_d[:])
            xp_t = cpool.tile([128, XP_COLS], dt.bfloat16, name="xp_t",
                              tag="xp_t")
            # one DMA per pair: progressive semaphores let the zigzag
            # M1-g0 phase start on pair 0 alone, ~4us before the full
            # xp transfer completes on this (slow) SWDGE queue
            for p in range(NPAIRS):
                nc.gpsimd.dma_start(
                    out=xp_t[:, p * PAIR_COLS:(p + 1) * PAIR_COLS],
                    in_=xp_d[:, p * PAIR_COLS:(p + 1) * PAIR_COLS])
            ind = bs_t[0:2, 0:128]  # parity indicator matrix (lhsT)

            # block-diagonal stationaries for the merged singles matmuls
            xs_t = [cpool.tile([128, XS_COLS], dt.bfloat16, name=f"xs{g}",
                               tag=f"xs{g}") for g in range(GROUPS)]
            for g in range(GROUPS):
                nc.vector.memset(xs_t[g][:], 0.0)
            # copyB (pair g) before copyA (pair g+1): the vector FIFO
            # then blocks on each pair's arrival in delivery order
            for g in range(GROUPS):
                # lower-right: row 2g+1 = pair g rip1 (kh=0 of row 2g+1)
                nc.vector.tensor_copy(
                    out=xs_t[g][64:128, :].rearrange(
                        "p (c m) -> p c m", m=128)[:, :, 64:128],
                    in_=xp_t[64:128, g * PAIR_COLS:
                             (g + 1) * PAIR_COLS].rearrange(
                        "p (c b) -> p c b", b=64))
                # upper-left: row 2g+2 = pair g+1 rip0 (kh=2 of row 2g)
                nc.vector.tensor_copy(
                    out=xs_t[g][0:64, :].rearrange(
                        "p (c m) -> p c m", m=128)[:, :, 0:64],
                    in_=xp_t[0:64, (g + 1) * PAIR_COLS:
                             (g + 2) * PAIR_COLS].rearrange(
                        "p (c b) -> p c b", b=64))

            ps = {}
            for g in range(GROUPS):
                for bk in range(BANKS):
                    ps[(g, bk)] = pspool.tile([128, 512], dt.float32,
                                              name=f"ps{g}{bk}", tag=f"ps{g}{bk}")
            out_sb = {(g, bk): cpool.tile([128, 512], dt.bfloat16,
                                          name=f"osb{g}{bk}", tag=f"osb{g}{bk}")
                      for g in range(GROUPS) for bk in range(BANKS)}

            # bias matmuls init psum (start=True): K=2 indicator trick
            # puts even-row bias on partitions 0:64, odd-row on 64:128.
            for g in range(GROUPS):
                for bk in range(BANKS):
                    a = 128 + (g * BANKS + bk) * 512
                    rhs = bs_t[0:2, a:a + 512]
                    nc.tensor.matmul(ps[(g, bk)][0:128, :], ind, rhs,
                                     start=True, stop=False)

            # all weight-chunk DMAs issue up front (fully SBUF-resident)
            wtiles = []
            for k, (a, b_) in enumerate(_CHUNKS):
                wt_k = cpool.tile([128, b_ - a], dt.float8e3,
                                  name=f"wtile{k}", tag=f"wt{k}")
                dma_eng = nc.sync if k % 2 == 0 else nc.scalar
                dma_eng.dma_start(
                    out=wt_k[:],
                    in_=wt_d[128 * a: 128 * b_].rearrange(
                        "(p n) -> p n", p=128))
                wtiles.append(wt_k)

            for idx, m in enumerate(mms):
                coff = _CHUNKS[m["chunk"]][0]
                wtile = wtiles[m["chunk"]]
                if m["src"] == "xp":
                    lhsT = xp_t[0:128, m["xoff"]:m["xoff"] + 64]
                else:
                    lhsT = xs_t[m["g"]][0:128, m["xoff"]:m["xoff"] + 128]
                rhs = wtile[0:128, m["c0"] - coff:m["c1"] - coff]
                if m["par"] == 2:
                    outap = ps[(m["g"], m["bk"])][0:128, m["o0"]:m["o1"]]
                else:
                    outap = ps[(m["g"], m["bk"])][
                        m["par"] * 64:(m["par"] + 1) * 64, m["o0"]:m["o1"]]
                nc.tensor.matmul(outap, lhsT, rhs, start=False,
                                 stop=m["stop"])
                if idx in evac_after:
                    g, bk = evac_after[idx]
                    nc.vector.tensor_copy(out=out_sb[(g, bk)][:],
                                          in_=ps[(g, bk)][:])
                    out_eng = nc.sync if (g * BANKS + bk) % 2 == 0 \
                        else nc.scalar
                    out_eng.dma_start(out=out_d[g, bk],
                                      in_=out_sb[(g, bk)][:])
    nc.compile()
    return nc


_NC_CACHE = [None]


def _get_nc():
    if _NC_CACHE[0] is None:
        _NC_CACHE[0] = build_nc()
    return _NC_CACHE[0]


def run_cores(in_maps, trace=False, **kw):
    nc = _get_nc()
    return run_bass_kernel_spmd(nc, in_maps, list(range(NCORES)),
                                trace=trace, **kw)


def unshard(results):
    y = np.empty((B, H, W, COUT), np.float32)
    for core in range(NCORES):
        o = np.asarray(results[core]["out"], np.float32)
        o = o.reshape(GROUPS, BANKS, 2, B, JPB, COUT)
        o = o.transpose(3, 0, 2, 1, 4, 5)  # [b, g, par, bk, j8, co]
        y[:, RPC * core: RPC * core + RPC] = o.reshape(B, RPC, W, COUT)
    return y


def kernel(inputs, kernel, bias):
    in_maps = prep_in_maps(inputs, kernel, bias)
    res = run_cores(in_maps)
    return unshard(res.results)


# revision 40
# speedup vs baseline: 1.1320x; 1.1320x over previous
"""Locally-connected conv (unshared weights) on 8 TRN2 NeuronCores.

Problem: inputs [64,32,32,64] f32, kernel [32,32,576,64] f32 (per-location
weights, KFEAT=3*3*64), bias [32,32,64] f32 -> out [64,32,32,64] f32
(SAME padding, stride 1).

Strategy (X-stationary, fp8e3 weight-streaming):
  - Spatial shard: core c computes output rows 4c..4c+3 (host slices the
    zero-padded input with halo; no device collectives needed).
  - Weights are unshared -> each weight element is used exactly once, so
    they are the *moving* matmul operand, streamed from HBM as fp8e3
    (e3m4) scaled by 2^5; X is bf16 scaled by 2^-5 so the product is at
    natural scale (measured end-to-end rel err ~1.35e-2 vs the 2e-2
    gate). This halves the dominant HBM traffic vs bf16 (9.2->4.6MB).
  - X patches are the *stationary* operand. K packs 2 input rows x 64
    channels = 128. xp holds only the 32 real columns (no pad cols).
  - Per output row pair (i0=2g, i1=2g+1) and input column c:
      M1: x pair g     (rows i0,i0+1), K=128 -> row i0 kh=(0,1)
      M2: x pair g+1   (rows i0+2,i0+3), K=128 -> row i1 kh=(1,2)
      MS: merged singles: block-diagonal stationary xs_g [128,128]
          (upper-left = row i0+2 for row i0's kh=2; lower-right =
          row i0+1 for row i1's kh=0; zeros elsewhere) -> ONE K=128
          matmul streams both rows' single-tap weights (two K=64
          matmuls would double-stream these columns through the PE).
    xs tiles are built on-device by the vector engine (memset + two
    strided copies from xp) while the first weight chunks stream in.
  - PSUM: bank tile [128,512] holds both rows of a group: even row on
    partitions 0:64, odd row on 64:128. All 4 output rows in 8 banks.
  - Bias is added with one K=2 bf16 matmul per PSUM bank (lhsT = parity
    indicator matrix) which also initializes the bank (start=True).
  - The whole fp8 weight stream (36KB/partition) is SBUF-resident:
    every chunk has its own buffer and all DMAs issue up front on the
    two HWDGE rings (sync/scalar), so the SDMA engines never starve;
    xp rides the gpsimd SWDGE ring in parallel.
"""

import numpy as np
import ml_dtypes

import concourse.bass as bass  # noqa: F401
import concourse.mybir as mybir
import concourse.tile as tile
from concourse import bacc
from concourse.bass_utils import run_bass_kernel_spmd

BF16 = ml_dtypes.bfloat16
E3M4 = ml_dtypes.float8_e3m4

B, H, W, CIN, COUT = 64, 32, 32, 64, 64
KH, KW = 3, 3
KFEAT = KH * KW * CIN
NCORES = 8
RPC = H // NCORES              # output rows per core = 4
HP, WP = H + 2, W + 2          # zero-padded input dims
NPAIRS = 3                     # input row pairs per core (6 padded rows)
PAIR_COLS = W * B              # 2048 free cols per pair tile (real cols only)
XP_COLS = NPAIRS * PAIR_COLS   # 6144
GROUPS = 2                     # output row pairs per core
BANKS = 4                      # psum banks per group
JPB = 8                        # output cols per bank (512 f32 / 64 co)
ROW_COLS = W * COUT            # 2048
XS_COLS = W * 128              # 4096: per-c [128,128] block-diag stationary
WSCALE = 32.0                  # weights *= 2^5 (fp8e3), x *= 2^-5
# bias stream [2, BS_COLS]: cols 0:128 = parity indicator matrix
# (row0 = 1 on m<64, row1 = 1 on m>=64), then per-(g,bank) [2,512] bias
# blocks (row0 = even-row bias, row1 = odd-row bias).
BS_COLS = 128 + GROUPS * BANKS * 512  # 4224


def stream_layout():
    """Weight stream block order. Returns (records, chunks, total_cols).

    record = (g, c, typ, jset, col_off); typ 0=M1, 1=M2, 2=MS(merged).
    chunks = list of (start_col, end_col), broken at (g,c) boundaries.
    """
    recs = []
    off = 0
    bounds = []
    for g in range(GROUPS):
        for phase_typs in ((0, 1), (2,)):
            for c in range(1, WP - 1):
                jset = [j for j in (c - 2, c - 1, c) if 0 <= j < W]
                if not jset:
                    continue
                for typ in phase_typs:
                    recs.append((g, c, typ, jset, off))
                    off += 64 * len(jset)
                bounds.append(off)
    chunks = []
    start, prev = 0, 0
    for b_ in bounds:
        cap = (1152, 2304)[len(chunks)] if len(chunks) < 2 else 4608
        if b_ - start > cap:
            chunks.append((start, prev))
            start = prev
        prev = b_
    chunks.append((start, prev))
    return recs, chunks, off


_RECS, _CHUNKS, TOTAL_COLS = stream_layout()


def mm_records():
    """Expand stream records into per-matmul records with psum targets."""
    chunk_of = {}
    for k, (a, b_) in enumerate(_CHUNKS):
        for g, c, typ, jset, off in _RECS:
            if a <= off < b_:
                chunk_of[off] = k
    mms = []
    for g, c, typ, jset, off in _RECS:
        # split jset (contiguous ascending) into per-bank pieces
        s = 0
        while s < len(jset):
            bk = jset[s] // JPB
            e = s
            while e < len(jset) and jset[e] // JPB == bk:
                e += 1
            c0 = off + s * 64
            c1 = off + e * 64
            o0 = (jset[s] % JPB) * 64
            o1 = o0 + (e - s) * 64
            if typ == 0:    # M1: row i0 (par 0), x pair g, K=128
                src, xoff, par = "xp", g * PAIR_COLS + (c - 1) * 64, 0
            elif typ == 1:  # M2: row i1 (par 1), x pair g+1, K=128
                src, xoff, par = "xp", (g + 1) * PAIR_COLS + (c - 1) * 64, 1
            else:           # MS: merged singles, block-diag xs_g, K=128
                src, xoff, par = "xs", (c - 1) * 128, 2
            mms.append(dict(g=g, bk=bk, src=src, xoff=xoff, par=par,
                            c0=c0, c1=c1, o0=o0, o1=o1,
                            chunk=chunk_of[off]))
            s = e
    return mms


_weight_template_cache = [None]


def weight_template():
    """int64 [128, TOTAL_COLS]: flat index into core-0 kernel array."""
    if _weight_template_cache[0] is not None:
        return _weight_template_cache[0]
    T = np.empty((128, TOTAL_COLS), np.int64)
    co = np.arange(COUT)
    p = np.arange(128)
    ci = p % 64
    for g, c, typ, jset, off in _RECS:
        for jj, j in enumerate(jset):
            kw = c - j
            if typ == 0:
                i = np.full(128, 2 * g)
                kh = np.where(p < 64, 0, 1)
            elif typ == 1:
                i = np.full(128, 2 * g + 1)
                kh = np.where(p < 64, 1, 2)
            else:
                i = np.where(p < 64, 2 * g, 2 * g + 1)
                kh = np.where(p < 64, 2, 0)
            # conv_general_dilated_local flattens KFEAT as (ci, kh, kw)
            kf = ci * (KH * KW) + kh * KW + kw
            base = ((i * W + j) * KFEAT + kf) * COUT
            T[:, off + jj * 64: off + (jj + 1) * 64] = base[:, None] + co[None, :]
    _weight_template_cache[0] = T
    return T


def prep_in_maps(inputs, kernel, bias):
    inputs = np.asarray(inputs, np.float32)
    kernel = np.asarray(kernel, np.float32)
    bias = np.asarray(bias, np.float32)
    T = weight_template()
    kflat = np.ascontiguousarray(kernel).reshape(-1)
    xpad = np.zeros((B, HP, W, CIN), np.float32)
    xpad[:, 1:H + 1, :, :] = inputs * (1.0 / WSCALE)
    xpad = xpad.astype(BF16)
    in_maps = []
    for core in range(NCORES):
        rows = xpad[:, RPC * core: RPC * core + 6]          # [B, 6, W, CIN]
        rt = rows.transpose(1, 3, 2, 0)                     # [r, ci, col, b]
        rt = rt.reshape(NPAIRS, 2, CIN, W, B).transpose(1, 2, 0, 3, 4)
        xp = np.ascontiguousarray(rt.reshape(128, XP_COLS))  # [rip*ci, rp,col,b]
        woff = (RPC * core) * W * KFEAT * COUT
        wt = np.clip(kflat[T + woff] * WSCALE, -15.5, 15.5).astype(E3M4)
        wt = np.concatenate([wt[:, a:b].reshape(-1) for a, b in _CHUNKS])
        bsh = bias[RPC * core: RPC * core + RPC].reshape(2 * GROUPS, ROW_COLS)
        bs = np.zeros((2, BS_COLS), np.float32)
        bs[0, 0:64] = 1.0
        bs[1, 64:128] = 1.0
        for g in range(GROUPS):
            for bk in range(BANKS):
                a = 128 + (g * BANKS + bk) * 512
                bs[0, a:a + 512] = bsh[2 * g, bk * 512:(bk + 1) * 512]
                bs[1, a:a + 512] = bsh[2 * g + 1, bk * 512:(bk + 1) * 512]
        in_maps.append({"xp": xp, "wt": wt, "bs": bs.astype(BF16)})
    return in_maps


def build_nc():
    dt = mybir.dt
    nc = bacc.Bacc(None, target_bir_lowering=False, debug=False)
    xp_d = nc.declare_dram_parameter("xp", [128, XP_COLS], dt.bfloat16,
                                     isOutput=False)
    wt_d = nc.declare_dram_parameter("wt", [128 * TOTAL_COLS], dt.float8e3,
                                     isOutput=False)
    bs_d = nc.declare_dram_parameter("bs", [2, BS_COLS], dt.bfloat16,
                                     isOutput=False)
    out_d = nc.declare_dram_parameter("out", [GROUPS, BANKS, 128, 512],
                                      dt.bfloat16, isOutput=True)

    mms = mm_records()
    for m in mms:
        m["stop"] = False
    last_bk = {}
    for idx, m in enumerate(mms):
        last_bk[(m["g"], m["bk"])] = idx
    for idx in last_bk.values():
        mms[idx]["stop"] = True
    evac_after = {idx: key for key, idx in last_bk.items()}

    with tile.TileContext(nc) as tc:
        with tc.tile_pool(name="const", bufs=1) as cpool, \
             tc.tile_pool(name="ps", bufs=1, space="PSUM") as pspool:
            bs_t = cpool.tile([2, BS_COLS], dt.bfloat16, name="bs_t",
                              tag="bs_t")
            nc.scalar.dma_start(out=bs_t[:], in_=bs_d[:])
            xp_t = cpool.tile([128, XP_COLS], dt.bfloat16, name="xp_t",
                              tag="xp_t")
            nc.gpsimd.dma_start(out=xp_t[:, 0:2 * PAIR_COLS],
                                in_=xp_d[:, 0:2 * PAIR_COLS])
            nc.gpsimd.dma_start(out=xp_t[:, 2 * PAIR_COLS:],
                                in_=xp_d[:, 2 * PAIR_COLS:])
            ind = bs_t[0:2, 0:128]  # parity indicator matrix (lhsT)

            # block-diagonal stationaries for the merged singles matmuls
            xs_t = [cpool.tile([128, XS_COLS], dt.bfloat16, name=f"xs{g}",
                               tag=f"xs{g}") for g in range(GROUPS)]
            for g in range(GROUPS):
                nc.vector.memset(xs_t[g][:], 0.0)
            for g in range(GROUPS):
                # upper-left: row 2g+2 = pair g+1 rip0 (kh=2 of row 2g)
                nc.vector.tensor_copy(
                    out=xs_t[g][0:64, :].rearrange(
                        "p (c m) -> p c m", m=128)[:, :, 0:64],
                    in_=xp_t[0:64, (g + 1) * PAIR_COLS:
                             (g + 2) * PAIR_COLS].rearrange(
                        "p (c b) -> p c b", b=64))
                # lower-right: row 2g+1 = pair g rip1 (kh=0 of row 2g+1)
                nc.vector.tensor_copy(
                    out=xs_t[g][64:128, :].rearrange(
                        "p (c m) -> p c m", m=128)[:, :, 64:128],
                    in_=xp_t[64:128, g * PAIR_COLS:
                             (g + 1) * PAIR_COLS].rearrange(
                        "p (c b) -> p c b", b=64))

            ps = {}
            for g in range(GROUPS):
                for bk in range(BANKS):
                    ps[(g, bk)] = pspool.tile([128, 512], dt.float32,
                                              name=f"ps{g}{bk}", tag=f"ps{g}{bk}")
            out_sb = {(g, bk): cpool.tile([128, 512], dt.bfloat16,
                                          name=f"osb{g}{bk}", tag=f"osb{g}{bk}")
                      for g in range(GROUPS) for bk in range(BANKS)}

            # bias matmuls init psum (start=True): K=2 indicator trick
            # puts even-row bias on partitions 0:64, odd-row on 64:128.
            for g in range(GROUPS):
                for bk in range(BANKS):
                    a = 128 + (g * BANKS + bk) * 512
                    rhs = bs_t[0:2, a:a + 512]
                    nc.tensor.matmul(ps[(g, bk)][0:128, :], ind, rhs,
                                     start=True, stop=False)

            # all weight-chunk DMAs issue up front (fully SBUF-resident)
            wtiles = []
            for k, (a, b_) in enumerate(_CHUNKS):
                wt_k = cpool.tile([128, b_ - a], dt.float8e3,
                                  name=f"wtile{k}", tag=f"wt{k}")
                dma_eng = nc.sync if k % 2 == 0 else nc.scalar
                dma_eng.dma_start(
                    out=wt_k[:],
                    in_=wt_d[128 * a: 128 * b_].rearrange(
                        "(p n) -> p n", p=128))
                wtiles.append(wt_k)

            for idx, m in enumerate(mms):
                coff = _CHUNKS[m["chunk"]][0]
                wtile = wtiles[m["chunk"]]
                if m["src"] == "xp":
                    lhsT = xp_t[0:128, m["xoff"]:m["xoff"] + 64]
                else:
                    lhsT = xs_t[m["g"]][0:128, m["xoff"]:m["xoff"] + 128]
                rhs = wtile[0:128, m["c0"] - coff:m["c1"] - coff]
                if m["par"] == 2:
                    outap = ps[(m["g"], m["bk"])][0:128, m["o0"]:m["o1"]]
                else:
                    outap = ps[(m["g"], m["bk"])][
                        m["par"] * 64:(m["par"] + 1) * 64, m["o0"]:m["o1"]]
                nc.tensor.matmul(outap, lhsT, rhs, start=False,
                                 stop=m["stop"])
                if idx in evac_after:
                    g, bk = evac_after[idx]
                    nc.vector.tensor_copy(out=out_sb[(g, bk)][:],
                                          in_=ps[(g, bk)][:])
                    nc.scalar.dma_start(out=out_d[g, bk],
                                        in_=out_sb[(g, bk)][:])
    nc.compile()
    return nc


_NC_CACHE = [None]


def _get_nc():
    if _NC_CACHE[0] is None:
        _NC_CACHE[0] = build_nc()
    return _NC_CACHE[0]


def run_cores(in_maps, trace=False, **kw):
    nc = _get_nc()
    return run_bass_kernel_spmd(nc, in_maps, list(range(NCORES)),
                                trace=trace, **kw)


def unshard(results):
    y = np.empty((B, H, W, COUT), np.float32)
    for core in range(NCORES):
        o = np.asarray(results[core]["out"], np.float32)
        o = o.reshape(GROUPS, BANKS, 2, B, JPB, COUT)
        o = o.transpose(3, 0, 2, 1, 4, 5)  # [b, g, par, bk, j8, co]
        y[:, RPC * core: RPC * core + RPC] = o.reshape(B, RPC, W, COUT)
    return y


def kernel(inputs, kernel, bias):
    in_maps = prep_in_maps(inputs, kernel, bias)
    res = run_cores(in_maps)
    return unshard(res.results)
